# revision 14
# baseline (speedup 1.0000x reference)
"""Causal self-attention (B=4, T=2048, C=1024, H=16) on 8 TRN2 NeuronCores.

Sharding: tensor-parallel over heads. Each core owns 2 of the 16 heads and
produces a partial (C, B*T) output (transposed); the host sums the 8 partials
and transposes back.

Key design points (vs. the earlier two-pass softmax kernel):
  - NO stats pass. The softmax shift is a host-precomputed, data-independent
    estimate m^(t) = sv(t) * sigma_h * sqrt(2 ln t), where sv(t) =
    log(t)^alpha/sqrt(D) is the position scale and sigma_h is the score std
    derived from the w_q/w_k column norms. On this problem's data the
    estimate sits within [-55, +43] of the true row max; storing P = exp(s -
    m^) in bf16 (range e^+-88) and accumulating in fp32 PSUM absorbs that
    slack. The shift cancels exactly between numerator and denominator.
  - The attention inner loop is PE-bound (S^T + PV matmuls ~2x the ACT exp
    time), so the PE never idles and the HAM clock gate stays at 2.4 GHz
    (the old kernel ran the whole attention phase throttled at 1.2 GHz).
  - exp is batched two k-tiles at a time over a [128,1024] 2-bank PSUM tile
    to amortize the ~350-cycle ACT instruction overhead.
  - v is computed transposed (stationary w_v, 8 matmuls/chunk instead of 32)
    then moved to natural [rows, feat] layout with PE transposes (the DMA
    XBAR transpose needs a 256B-aligned contiguous destination, which the
    [.., 65]-strided vA layout can't provide).
  - c_proj is emitted output-transposed (stationary w_proj tiles), writing
    outT [C, B*T]; the host transposes after the cross-core sum.

Layout (per core):
  - xT [C, BT] fp16 and xsT (pre-scaled by sv(t)) land contraction-major.
  - qsT [65, pair, T]: rows 0-63 = q' = w_q^T(sv*x), row 64 = -m^(t).
  - kaT [65, pair, T]: rows 0-63 = k, row 64 = ones. S^T = kaT^T @ qsT gives
    scores with the shift already applied (65-dim augmented contraction).
  - vA [128, pair, kt, 65] bf16: v natural + ones column; PV with moving
    P^T produces y^T plus the softmax denominator in row 64.
"""

import sys

if "/opt/trn_rl_repo" not in sys.path:
    sys.path.insert(0, "/opt/trn_rl_repo")

import math

import numpy as np

# ---------------------------------------------------------------- constants
B, T, C, H, D = 4, 2048, 1024, 16, 64
ALPHA = 2.0
NCORES = 8
HPC = H // NCORES          # heads per core = 2
NP = B * HPC               # (batch, head) pairs per core = 8
BT = B * T                 # 8192 rows
KC = C // 128              # 8 contraction tiles for the qkv projection
CH = 512                   # row chunk / query strip width
NCH = BT // CH             # 16 chunks
QTPB = T // 128            # 16 k-tiles per batch
SPB = T // CH              # 4 query strips per batch
NEG = -1.0e9

_F16 = np.float16


def _build_nc():
    import concourse.mybir as mybir
    from concourse import bacc
    from concourse.masks import make_identity
    from concourse.tile import TileContext

    f16 = mybir.dt.float16
    bf16 = mybir.dt.bfloat16
    f32 = mybir.dt.float32
    EXP = mybir.ActivationFunctionType.Exp

    nc = bacc.Bacc()

    xT = nc.dram_tensor("xT", [C, BT], f16, kind="ExternalInput")
    xsT = nc.dram_tensor("xsT", [C, BT], f16, kind="ExternalInput")
    wq = nc.dram_tensor("wq", [C, HPC * D], f16, kind="ExternalInput")
    wk = nc.dram_tensor("wk", [C, HPC * D], f16, kind="ExternalInput")
    wv = nc.dram_tensor("wv", [C, HPC * D], f16, kind="ExternalInput")
    wp = nc.dram_tensor("wp", [HPC * D, C], f16, kind="ExternalInput")
    mneg = nc.dram_tensor("mneg", [NP, T], f16, kind="ExternalInput")
    outT = nc.dram_tensor("outT", [C, BT], f16, kind="ExternalOutput")

    with TileContext(nc) as tc:
        with (
            tc.tile_pool(name="persist", bufs=1) as pp,
            tc.tile_pool(name="xin", bufs=2) as xp,
            tc.tile_pool(name="ptile", bufs=3) as ptp,
            tc.tile_pool(name="small", bufs=2) as sp,
            tc.tile_pool(name="ps2", bufs=2, space="PSUM") as ps2,
            tc.tile_pool(name="ps1", bufs=2, space="PSUM") as ps1,
        ):
            # ---- persistent tiles
            qsT = pp.tile([65, NP, T], f16, tag="qsT")          # q' + (-m^) row
            kaT = pp.tile([65, NP, T], f16, tag="kaT")          # k + ones row
            vA = pp.tile([128, NP, QTPB, 65], bf16, tag="vA")   # v + ones col
            yT = pp.tile([128, BT], f16, tag="yT")              # y^T, 2 heads
            wqs = pp.tile([128, KC, 128], f16, tag="wqs")
            wks = pp.tile([128, KC, 128], f16, tag="wks")
            wvs = pp.tile([128, KC, 128], f16, tag="wvs")
            wps = pp.tile([128, C], f16, tag="wps")
            maskK = pp.tile([128, 128], f32, tag="maskK")       # [k,q]: NEG if q<k
            ones1 = pp.tile([1, 64], bf16, tag="ones1")
            identB = pp.tile([128, 128], bf16, tag="identB")

            # ---- init constants
            nc.sync.dma_start(out=wqs, in_=wq[:, :].rearrange("(kt p) n -> p kt n", p=128))
            nc.sync.dma_start(out=wks, in_=wk[:, :].rearrange("(kt p) n -> p kt n", p=128))
            nc.sync.dma_start(out=wvs, in_=wv[:, :].rearrange("(kt p) n -> p kt n", p=128))
            nc.sync.dma_start(out=wps, in_=wp[:, :])
            idx = pp.tile([128, 128], mybir.dt.int32, tag="idx")
            nc.gpsimd.iota(idx, pattern=[[1, 128]], base=0, channel_multiplier=-1)
            nc.vector.tensor_scalar(
                out=maskK, in0=idx, scalar1=0, scalar2=float(NEG),
                op0=mybir.AluOpType.is_lt, op1=mybir.AluOpType.mult)
            nc.vector.memset(ones1, 1.0)
            make_identity(nc, identB)
            nc.vector.memset(vA[:, :, :, 64:65], 1.0)
            nc.vector.memset(kaT[64:65, :, :], 1.0)
            nc.sync.dma_start(out=qsT[64:65, :, :], in_=mneg[:, :])

            # ---- stage A: q/k/v^T projections per 512-row chunk
            def emit_chunk(n):
                b, loc = n // SPB, (n % SPB) * CH
                koff = (n % SPB) * (CH // 128)
                xt = xp.tile([128, KC, CH], f16, tag="xt")
                nc.sync.dma_start(
                    out=xt,
                    in_=xT[:, n * CH:(n + 1) * CH].rearrange(
                        "(kt p) r -> p kt r", p=128))
                xs = xp.tile([128, KC, CH], f16, tag="xs")
                nc.sync.dma_start(
                    out=xs,
                    in_=xsT[:, n * CH:(n + 1) * CH].rearrange(
                        "(kt p) r -> p kt r", p=128))
                qk = ps2.tile([128, 2 * CH], f32, tag="s")
                for kt in range(KC):
                    nc.tensor.matmul(qk[:, 0:CH], wqs[:, kt, :], xs[:, kt, :],
                                     start=(kt == 0), stop=(kt == KC - 1))
                for kt in range(KC):
                    nc.tensor.matmul(qk[:, CH:2 * CH], wks[:, kt, :], xt[:, kt, :],
                                     start=(kt == 0), stop=(kt == KC - 1))
                psv = ps1.tile([128, CH], f32, tag="po")
                for kt in range(KC):
                    nc.tensor.matmul(psv, wvs[:, kt, :], xt[:, kt, :],
                                     start=(kt == 0), stop=(kt == KC - 1))
                for h in range(HPC):
                    pair = b * HPC + h
                    nc.vector.tensor_copy(
                        qsT[0:64, pair, loc:loc + CH],
                        qk[h * 64:(h + 1) * 64, 0:CH])
                    nc.scalar.copy(
                        kaT[0:64, pair, loc:loc + CH],
                        qk[h * 64:(h + 1) * 64, CH:2 * CH])
                vt = sp.tile([128, CH], bf16, tag="vt")
                nc.vector.tensor_copy(vt, psv)
                for j in range(CH // 128):
                    tp = ps1.tile([128, 128], bf16, tag="y", name="tp")
                    nc.tensor.transpose(tp, vt[:, j * 128:(j + 1) * 128], identB)
                    nc.vector.tensor_copy(
                        vA[:, b * HPC:(b + 1) * HPC, koff + j, 0:64],
                        tp[:, :].rearrange("p (h d) -> p h d", h=HPC))

            # ---- attention: strip of 512 queries, 2-kt-batched exp pipeline.
            # The normalize chain of each strip is emitted lazily, interleaved
            # into the NEXT strip's g-loop, so the dbc matmul never sits at
            # the head of the PE queue waiting on the DVE reciprocal (that
            # stall re-throttled the HAM clock gate every strip).
            pending = []        # queued closures: one emission step each

            def drain_pending(k=1):
                if len(pending) > 6:
                    k += 1
                for _ in range(k):
                    if pending:
                        pending.pop(0)()

            def emit_strip(p, qs):
                b, h = p // HPC, p % HPC
                kts = 4 * (qs + 1)
                y_ps = ps1.tile([128, CH], f32, tag="y")
                psSs = {}
                pts = {}

                def offof(kt):
                    return max(0, kt * 128 - qs * CH)

                def emit_S(kt):
                    g, half = kt // 2, kt % 2
                    if half == 0:
                        psSs[g] = ps2.tile([128, 2 * CH], f32, tag="s",
                                           name="psS")
                    psS = psSs[g]
                    off = offof(kt)
                    lo = half * CH + off
                    nc.tensor.matmul(
                        psS[:, lo:(half + 1) * CH],
                        kaT[0:65, p, kt * 128:(kt + 1) * 128],
                        qsT[0:65, p, qs * CH + off:(qs + 1) * CH],
                        start=True, stop=True)
                    if kt >= 4 * qs:
                        nc.vector.tensor_add(
                            psS[:, lo:lo + 128], psS[:, lo:lo + 128], maskK)

                def emit_exp(g):
                    lo = offof(2 * g)
                    pt = ptp.tile([128, 2 * CH], bf16, tag="pt", name="pt")
                    pts[g] = pt
                    nc.scalar.activation(
                        pt[:, lo:2 * CH], psSs[g][:, lo:2 * CH], EXP)

                def emit_PV(kt):
                    g, half = kt // 2, kt % 2
                    off = offof(kt)
                    nc.tensor.matmul(
                        y_ps[0:65, off:CH],
                        vA[:, p, kt, :],
                        pts[g][:, half * CH + off:(half + 1) * CH],
                        start=(kt == 0), stop=(kt == kts - 1))

                gs = kts // 2
                for g in range(gs):
                    emit_S(2 * g)
                    emit_S(2 * g + 1)
                    emit_exp(g)
                    if g > 0:
                        emit_PV(2 * g - 2)
                        emit_PV(2 * g - 1)
                    drain_pending()
                emit_PV(kts - 2)
                emit_PV(kts - 1)

                # normalize: yT[h, rows] = y / denom (denom = row 64)
                state = {}

                def n_rec():
                    # copy denom to SBUF first: the approx reciprocal's
                    # BITWISE_NOT seed must see IEEE fp32 bits, not PSUM's
                    # internal accumulator format
                    dns = sp.tile([1, CH], f32, tag="dns", bufs=3, name="dns")
                    nc.vector.tensor_copy(dns, y_ps[64:65, :])
                    recs = sp.tile([1, CH], f32, tag="recs", bufs=3,
                                   name="recs")
                    nc.vector.reciprocal_approx_fast(out=recs, in_=dns)
                    recb = sp.tile([1, CH], bf16, tag="recb", bufs=3,
                                   name="recb")
                    nc.vector.tensor_copy(recb, recs)
                    state["recb"] = recb

                def n_dbc():
                    dbc = ps1.tile([128, CH], f32, tag="po", name="dbc")
                    nc.tensor.matmul(dbc[0:64, :], ones1, state["recb"],
                                     start=True, stop=True)
                    state["dbc"] = dbc

                def n_mul():
                    dsb = sp.tile([64, CH], bf16, tag="dsb", bufs=3,
                                  name="dsb")
                    nc.vector.tensor_copy(dsb, state["dbc"][0:64, :])
                    nc.vector.tensor_mul(
                        yT[h * 64:(h + 1) * 64,
                           b * T + qs * CH:b * T + (qs + 1) * CH],
                        y_ps[0:64, :], dsb)

                pending.extend([n_rec, n_dbc, n_mul])

            # ---- c_proj, output-transposed: outT[C, rows] per batch.
            # Each (nt, rs) unit becomes a pending-queue closure, so the proj
            # matmuls are spread through later strips as PE filler that keeps
            # the HAM clock gate warm.
            def queue_proj(b):
                def unit(nt, rs):
                    def run():
                        po = ps1.tile([128, CH], f32, tag="po", name="po")
                        nc.tensor.matmul(
                            po, wps[:, nt * 128:(nt + 1) * 128],
                            yT[:, b * T + rs * CH:b * T + (rs + 1) * CH],
                            start=True, stop=True)
                        o16 = ptp.tile([128, CH], f16, tag="o16", name="o16")
                        # split the PSUM->SBUF copy across ACT and DVE so the
                        # po slot frees in half the time (the PE's next proj
                        # matmul WAR-waits on this release)
                        nc.scalar.copy(o16[:, 0:CH // 2], po[:, 0:CH // 2])
                        nc.vector.tensor_copy(o16[:, CH // 2:CH],
                                              po[:, CH // 2:CH])
                        nc.sync.dma_start(
                            out=outT[nt * 128:(nt + 1) * 128,
                                     b * T + rs * CH:b * T + (rs + 1) * CH],
                            in_=o16)
                    return run
                for nt in range(C // 128):
                    for rs in range(SPB):
                        pending.append(unit(nt, rs))

            for n in range(NCH):
                emit_chunk(n)
            for p in range(NP):
                for qs in range(SPB):
                    emit_strip(p, qs)
                # queue batch b's proj once both its pairs are done; the
                # normalize closures of those pairs are already ahead of it
                # in the pending FIFO
                if p % HPC == HPC - 1:
                    queue_proj(p // HPC)
            drain_pending(len(pending))
    nc.compile()
    return nc


_NC_CACHE = None
TRACE = False           # set by test harness for profiling runs
LAST_RESULT = None      # BassKernelResults of the last run (when TRACE)


def kernel(x, w_attn, w_proj):
    global _NC_CACHE, LAST_RESULT
    from concourse.bass_utils import run_bass_kernel_spmd

    if _NC_CACHE is None:
        _NC_CACHE = _build_nc()
    nc = _NC_CACHE

    x2 = np.asarray(x, dtype=np.float32).reshape(BT, C)
    pos = np.arange(1, T + 1, dtype=np.float64)
    sv = (np.log(pos) ** ALPHA / math.sqrt(D)).astype(np.float64)
    sfull = np.tile(sv.astype(np.float32), B)
    xTm = np.ascontiguousarray(x2.T).astype(_F16)
    xsTm = np.ascontiguousarray((x2 * sfull[:, None]).T).astype(_F16)
    wa = np.asarray(w_attn, dtype=np.float32)
    wpj = np.asarray(w_proj, dtype=np.float32)

    # data-independent softmax shift: m^(h, t) = sv(t) * sigma_h * g(t)
    nq = (wa[:, :C] ** 2).sum(axis=0)
    nk = (wa[:, C:2 * C] ** 2).sum(axis=0)
    sigma = np.sqrt((nq * nk).reshape(H, D).sum(axis=1))      # (H,)
    g = np.sqrt(2.0 * np.log(np.clip(pos, 2.0, None)))        # (T,)

    in_maps = []
    for c in range(NCORES):
        h0 = c * HPC
        cols = np.r_[h0 * D:(h0 + HPC) * D]
        mn = np.empty((NP, T), dtype=np.float64)
        for pair in range(NP):
            hg = h0 + pair % HPC
            mn[pair] = -(sv * sigma[hg] * g)
        in_maps.append({
            "xT": xTm,
            "xsT": xsTm,
            "wq": np.ascontiguousarray(wa[:, cols]).astype(_F16),
            "wk": np.ascontiguousarray(wa[:, C + cols]).astype(_F16),
            "wv": np.ascontiguousarray(wa[:, 2 * C + cols]).astype(_F16),
            "wp": np.ascontiguousarray(wpj[cols, :]).astype(_F16),
            "mneg": mn.astype(_F16),
        })

    res = run_bass_kernel_spmd(
        nc, in_maps, core_ids=list(range(NCORES)), trace=TRACE)
    LAST_RESULT = res
    total = np.zeros((C, BT), dtype=np.float32)
    for r in res.results:
        total += r["outT"].astype(np.float32)
    return np.ascontiguousarray(total.T).reshape(B, T, C)


# revision 18
# speedup vs baseline: 1.1184x; 1.1184x over previous
"""Causal self-attention (B=4, T=2048, C=1024, H=16) on 8 TRN2 NeuronCores.

Sharding: tensor-parallel over heads. Each core owns 2 of the 16 heads and
produces a partial (C, B*T) output (transposed); the host sums the 8 partials
and transposes back.

Key design points (vs. the earlier two-pass softmax kernel):
  - NO stats pass. The softmax shift is a host-precomputed, data-independent
    estimate m^(t) = sv(t) * sigma_h * sqrt(2 ln t), where sv(t) =
    log(t)^alpha/sqrt(D) is the position scale and sigma_h is the score std
    derived from the w_q/w_k column norms. On this problem's data the
    estimate sits within [-55, +43] of the true row max; storing P = exp(s -
    m^) in bf16 (range e^+-88) and accumulating in fp32 PSUM absorbs that
    slack. The shift cancels exactly between numerator and denominator.
  - The attention inner loop is PE-bound (S^T + PV matmuls ~2x the ACT exp
    time), so the PE never idles and the HAM clock gate stays at 2.4 GHz
    (the old kernel ran the whole attention phase throttled at 1.2 GHz).
  - exp is batched two k-tiles at a time over a [128,1024] 2-bank PSUM tile
    to amortize the ~350-cycle ACT instruction overhead.
  - v is computed transposed (stationary w_v, 8 matmuls/chunk instead of 32)
    then moved to natural [rows, feat] layout with PE transposes (the DMA
    XBAR transpose needs a 256B-aligned contiguous destination, which the
    [.., 65]-strided vA layout can't provide).
  - c_proj is emitted output-transposed (stationary w_proj tiles), writing
    outT [C, B*T]; the host transposes after the cross-core sum.

Layout (per core):
  - xT [C, BT] fp16 and xsT (pre-scaled by sv(t)) land contraction-major.
  - qsT [65, pair, T]: rows 0-63 = q' = w_q^T(sv*x), row 64 = -m^(t).
  - kaT [65, pair, T]: rows 0-63 = k, row 64 = ones. S^T = kaT^T @ qsT gives
    scores with the shift already applied (65-dim augmented contraction).
  - vA [128, pair, kt, 65] bf16: v natural + ones column; PV with moving
    P^T produces y^T plus the softmax denominator in row 64.
"""

import sys

if "/opt/trn_rl_repo" not in sys.path:
    sys.path.insert(0, "/opt/trn_rl_repo")

import math

import numpy as np

# ---------------------------------------------------------------- constants
B, T, C, H, D = 4, 2048, 1024, 16, 64
ALPHA = 2.0
NCORES = 8
HPC = H // NCORES          # heads per core = 2
NP = B * HPC               # (batch, head) pairs per core = 8
BT = B * T                 # 8192 rows
KC = C // 128              # 8 contraction tiles for the qkv projection
CH = 512                   # row chunk / query strip width
NCH = BT // CH             # 16 chunks
QTPB = T // 128            # 16 k-tiles per batch
SPB = T // CH              # 4 query strips per batch
NEG = -1.0e9

_F16 = np.float16


def _build_nc():
    import concourse.mybir as mybir
    from concourse import bacc
    from concourse.masks import make_identity
    from concourse.tile import TileContext

    f16 = mybir.dt.float16
    bf16 = mybir.dt.bfloat16
    f32 = mybir.dt.float32
    EXP = mybir.ActivationFunctionType.Exp

    nc = bacc.Bacc()

    xT = nc.dram_tensor("xT", [C, BT], f16, kind="ExternalInput")
    xsT = nc.dram_tensor("xsT", [C, BT], f16, kind="ExternalInput")
    wq = nc.dram_tensor("wq", [C, HPC * D], f16, kind="ExternalInput")
    wk = nc.dram_tensor("wk", [C, HPC * D], f16, kind="ExternalInput")
    wv = nc.dram_tensor("wv", [C, HPC * D], f16, kind="ExternalInput")
    wp = nc.dram_tensor("wp", [HPC * D, C], f16, kind="ExternalInput")
    mneg = nc.dram_tensor("mneg", [NP, T], f16, kind="ExternalInput")
    outT = nc.dram_tensor("outT", [C, BT], f16, kind="ExternalOutput")

    with TileContext(nc) as tc:
        with (
            tc.tile_pool(name="persist", bufs=1) as pp,
            tc.tile_pool(name="xin", bufs=2) as xp,
            tc.tile_pool(name="ptile", bufs=3) as ptp,
            tc.tile_pool(name="small", bufs=2) as sp,
            tc.tile_pool(name="ps2", bufs=2, space="PSUM") as ps2,
            tc.tile_pool(name="ps1", bufs=2, space="PSUM") as ps1,
        ):
            # ---- persistent tiles
            qsT = pp.tile([65, NP, T], f16, tag="qsT")          # q' + (-m^) row
            kaT = pp.tile([65, NP, T], f16, tag="kaT")          # k + ones row
            vA = pp.tile([128, NP, QTPB, 65], bf16, tag="vA")   # v + ones col
            yT = pp.tile([128, BT], f16, tag="yT")              # y^T, 2 heads
            wqs = pp.tile([128, KC, 128], f16, tag="wqs")
            wks = pp.tile([128, KC, 128], f16, tag="wks")
            wvs = pp.tile([128, KC, 128], f16, tag="wvs")
            wps = pp.tile([128, C], f16, tag="wps")
            maskK = pp.tile([128, 128], f32, tag="maskK")       # [k,q]: NEG if q<k
            ones1 = pp.tile([1, 64], bf16, tag="ones1")
            identB = pp.tile([128, 128], bf16, tag="identB")

            # ---- init constants
            nc.sync.dma_start(out=wqs, in_=wq[:, :].rearrange("(kt p) n -> p kt n", p=128))
            nc.sync.dma_start(out=wks, in_=wk[:, :].rearrange("(kt p) n -> p kt n", p=128))
            nc.sync.dma_start(out=wvs, in_=wv[:, :].rearrange("(kt p) n -> p kt n", p=128))
            nc.sync.dma_start(out=wps, in_=wp[:, :])
            idx = pp.tile([128, 128], mybir.dt.int32, tag="idx")
            nc.gpsimd.iota(idx, pattern=[[1, 128]], base=0, channel_multiplier=-1)
            nc.vector.tensor_scalar(
                out=maskK, in0=idx, scalar1=0, scalar2=float(NEG),
                op0=mybir.AluOpType.is_lt, op1=mybir.AluOpType.mult)
            nc.vector.memset(ones1, 1.0)
            make_identity(nc, identB)
            nc.vector.memset(vA[:, :, :, 64:65], 1.0)
            nc.vector.memset(kaT[64:65, :, :], 1.0)
            nc.sync.dma_start(out=qsT[64:65, :, :], in_=mneg[:, :])

            # ---- stage A: q/k/v^T projections per 512-row chunk
            def emit_chunk(n):
                b, loc = n // SPB, (n % SPB) * CH
                koff = (n % SPB) * (CH // 128)
                xt = xp.tile([128, KC, CH], f16, tag="xt")
                nc.sync.dma_start(
                    out=xt,
                    in_=xT[:, n * CH:(n + 1) * CH].rearrange(
                        "(kt p) r -> p kt r", p=128))
                xs = xp.tile([128, KC, CH], f16, tag="xs")
                nc.sync.dma_start(
                    out=xs,
                    in_=xsT[:, n * CH:(n + 1) * CH].rearrange(
                        "(kt p) r -> p kt r", p=128))
                qk = ps2.tile([128, 2 * CH], f32, tag="s")
                for kt in range(KC):
                    nc.tensor.matmul(qk[:, 0:CH], wqs[:, kt, :], xs[:, kt, :],
                                     start=(kt == 0), stop=(kt == KC - 1))
                for kt in range(KC):
                    nc.tensor.matmul(qk[:, CH:2 * CH], wks[:, kt, :], xt[:, kt, :],
                                     start=(kt == 0), stop=(kt == KC - 1))
                psv = ps1.tile([128, CH], f32, tag="po")
                for kt in range(KC):
                    nc.tensor.matmul(psv, wvs[:, kt, :], xt[:, kt, :],
                                     start=(kt == 0), stop=(kt == KC - 1))
                for h in range(HPC):
                    pair = b * HPC + h
                    nc.vector.tensor_copy(
                        qsT[0:64, pair, loc:loc + CH],
                        qk[h * 64:(h + 1) * 64, 0:CH])
                    nc.scalar.copy(
                        kaT[0:64, pair, loc:loc + CH],
                        qk[h * 64:(h + 1) * 64, CH:2 * CH])
                vt = sp.tile([128, CH], bf16, tag="vt")
                nc.vector.tensor_copy(vt, psv)
                for j in range(CH // 128):
                    tp = ps1.tile([128, 128], bf16, tag="y", name="tp")
                    nc.tensor.transpose(tp, vt[:, j * 128:(j + 1) * 128], identB)
                    nc.vector.tensor_copy(
                        vA[:, b * HPC:(b + 1) * HPC, koff + j, 0:64],
                        tp[:, :].rearrange("p (h d) -> p h d", h=HPC))

            # ---- attention: strip of 512 queries, 2-kt-batched exp pipeline.
            # The normalize chain of each strip is emitted lazily, interleaved
            # into the NEXT strip's g-loop, so the dbc matmul never sits at
            # the head of the PE queue waiting on the DVE reciprocal (that
            # stall re-throttled the HAM clock gate every strip).
            # Two deferred-emission queues. Normalize closures release y_ps
            # slots that the next strips WAR-wait on, so they drain fully at
            # every g-step; proj units are pure filler and trickle 1-2 per
            # step so they never dam the normalize chain.
            pending_norm = []
            pending_proj = []

            def drain_pending(k=1):
                while pending_norm:
                    pending_norm.pop(0)()
                if len(pending_proj) > 16:
                    k += 1
                for _ in range(k):
                    if pending_proj:
                        pending_proj.pop(0)()

            def emit_strip(p, qs):
                b, h = p // HPC, p % HPC
                kts = 4 * (qs + 1)
                y_ps = ps1.tile([128, CH], f32, tag="y")
                psSs = {}
                pts = {}

                def offof(kt):
                    return max(0, kt * 128 - qs * CH)

                def emit_S(kt):
                    g, half = kt // 2, kt % 2
                    if half == 0:
                        psSs[g] = ps2.tile([128, 2 * CH], f32, tag="s",
                                           name="psS")
                    psS = psSs[g]
                    off = offof(kt)
                    lo = half * CH + off
                    nc.tensor.matmul(
                        psS[:, lo:(half + 1) * CH],
                        kaT[0:65, p, kt * 128:(kt + 1) * 128],
                        qsT[0:65, p, qs * CH + off:(qs + 1) * CH],
                        start=True, stop=True)
                    if kt >= 4 * qs:
                        nc.vector.tensor_add(
                            psS[:, lo:lo + 128], psS[:, lo:lo + 128], maskK)

                def emit_exp(g):
                    lo = offof(2 * g)
                    pt = ptp.tile([128, 2 * CH], bf16, tag="pt", name="pt")
                    pts[g] = pt
                    nc.scalar.activation(
                        pt[:, lo:2 * CH], psSs[g][:, lo:2 * CH], EXP)

                def emit_PV(kt):
                    g, half = kt // 2, kt % 2
                    off = offof(kt)
                    nc.tensor.matmul(
                        y_ps[0:65, off:CH],
                        vA[:, p, kt, :],
                        pts[g][:, half * CH + off:(half + 1) * CH],
                        start=(kt == 0), stop=(kt == kts - 1))

                gs = kts // 2
                for g in range(gs):
                    emit_S(2 * g)
                    emit_S(2 * g + 1)
                    emit_exp(g)
                    if g > 0:
                        emit_PV(2 * g - 2)
                        emit_PV(2 * g - 1)
                    drain_pending()
                emit_PV(kts - 2)
                emit_PV(kts - 1)

                # normalize: yT[h, rows] = y / denom (denom = row 64)
                state = {}

                def n_rec():
                    # copy denom to SBUF first: the approx reciprocal's
                    # BITWISE_NOT seed must see IEEE fp32 bits, not PSUM's
                    # internal accumulator format
                    dns = sp.tile([1, CH], f32, tag="dns", bufs=3, name="dns")
                    nc.vector.tensor_copy(dns, y_ps[64:65, :])
                    recs = sp.tile([1, CH], f32, tag="recs", bufs=3,
                                   name="recs")
                    nc.vector.reciprocal_approx_fast(out=recs, in_=dns)
                    recb = sp.tile([1, CH], bf16, tag="recb", bufs=3,
                                   name="recb")
                    nc.vector.tensor_copy(recb, recs)
                    state["recb"] = recb

                def n_dbc():
                    dbc = ps1.tile([128, CH], f32, tag="po", name="dbc")
                    nc.tensor.matmul(dbc[0:64, :], ones1, state["recb"],
                                     start=True, stop=True)
                    state["dbc"] = dbc

                def n_mul():
                    dsb = sp.tile([64, CH], bf16, tag="dsb", bufs=3,
                                  name="dsb")
                    nc.vector.tensor_copy(dsb, state["dbc"][0:64, :])
                    nc.vector.tensor_mul(
                        yT[h * 64:(h + 1) * 64,
                           b * T + qs * CH:b * T + (qs + 1) * CH],
                        y_ps[0:64, :], dsb)

                pending_norm.extend([n_rec, n_dbc, n_mul])

            # ---- c_proj, output-transposed: outT[C, rows] per batch.
            # Each (nt, rs) unit becomes a pending-queue closure, so the proj
            # matmuls are spread through later strips as PE filler that keeps
            # the HAM clock gate warm.
            def queue_proj(b):
                def unit(nt, rs):
                    def run():
                        po = ps1.tile([128, CH], f32, tag="po", name="po")
                        nc.tensor.matmul(
                            po, wps[:, nt * 128:(nt + 1) * 128],
                            yT[:, b * T + rs * CH:b * T + (rs + 1) * CH],
                            start=True, stop=True)
                        o16 = ptp.tile([128, CH], f16, tag="o16", name="o16")
                        # split the PSUM->SBUF copy across ACT and DVE so the
                        # po slot frees in half the time (the PE's next proj
                        # matmul WAR-waits on this release)
                        nc.scalar.copy(o16[:, 0:CH // 2], po[:, 0:CH // 2])
                        nc.vector.tensor_copy(o16[:, CH // 2:CH],
                                              po[:, CH // 2:CH])
                        nc.sync.dma_start(
                            out=outT[nt * 128:(nt + 1) * 128,
                                     b * T + rs * CH:b * T + (rs + 1) * CH],
                            in_=o16)
                    return run
                for nt in range(C // 128):
                    for rs in range(SPB):
                        pending_proj.append(unit(nt, rs))

            for n in range(NCH):
                emit_chunk(n)
            for p in range(NP):
                for qs in range(SPB):
                    emit_strip(p, qs)
                # queue batch b's proj once both its pairs are done; the
                # normalize closures of those pairs are already ahead of it
                # in the pending FIFO
                if p % HPC == HPC - 1:
                    queue_proj(p // HPC)
            drain_pending(len(pending_proj))
    nc.compile()
    return nc


_NC_CACHE = None
TRACE = False           # set by test harness for profiling runs
LAST_RESULT = None      # BassKernelResults of the last run (when TRACE)


def kernel(x, w_attn, w_proj):
    global _NC_CACHE, LAST_RESULT
    from concourse.bass_utils import run_bass_kernel_spmd

    if _NC_CACHE is None:
        _NC_CACHE = _build_nc()
    nc = _NC_CACHE

    x2 = np.asarray(x, dtype=np.float32).reshape(BT, C)
    pos = np.arange(1, T + 1, dtype=np.float64)
    sv = (np.log(pos) ** ALPHA / math.sqrt(D)).astype(np.float64)
    sfull = np.tile(sv.astype(np.float32), B)
    xTm = np.ascontiguousarray(x2.T).astype(_F16)
    xsTm = np.ascontiguousarray((x2 * sfull[:, None]).T).astype(_F16)
    wa = np.asarray(w_attn, dtype=np.float32)
    wpj = np.asarray(w_proj, dtype=np.float32)

    # data-independent softmax shift: m^(h, t) = sv(t) * sigma_h * g(t)
    nq = (wa[:, :C] ** 2).sum(axis=0)
    nk = (wa[:, C:2 * C] ** 2).sum(axis=0)
    sigma = np.sqrt((nq * nk).reshape(H, D).sum(axis=1))      # (H,)
    g = np.sqrt(2.0 * np.log(np.clip(pos, 2.0, None)))        # (T,)

    in_maps = []
    for c in range(NCORES):
        h0 = c * HPC
        cols = np.r_[h0 * D:(h0 + HPC) * D]
        mn = np.empty((NP, T), dtype=np.float64)
        for pair in range(NP):
            hg = h0 + pair % HPC
            mn[pair] = -(sv * sigma[hg] * g)
        in_maps.append({
            "xT": xTm,
            "xsT": xsTm,
            "wq": np.ascontiguousarray(wa[:, cols]).astype(_F16),
            "wk": np.ascontiguousarray(wa[:, C + cols]).astype(_F16),
            "wv": np.ascontiguousarray(wa[:, 2 * C + cols]).astype(_F16),
            "wp": np.ascontiguousarray(wpj[cols, :]).astype(_F16),
            "mneg": mn.astype(_F16),
        })

    res = run_bass_kernel_spmd(
        nc, in_maps, core_ids=list(range(NCORES)), trace=TRACE)
    LAST_RESULT = res
    total = np.zeros((C, BT), dtype=np.float32)
    for r in res.results:
        total += r["outT"].astype(np.float32)
    return np.ascontiguousarray(total.T).reshape(B, T, C)


# revision 20
# speedup vs baseline: 1.1888x; 1.0630x over previous
"""Causal self-attention (B=4, T=2048, C=1024, H=16) on 8 TRN2 NeuronCores.

Sharding: tensor-parallel over heads. Each core owns 2 of the 16 heads and
produces a partial (C, B*T) output (transposed); the host sums the 8 partials
and transposes back.

Key design points (vs. the earlier two-pass softmax kernel):
  - NO stats pass. The softmax shift is a host-precomputed, data-independent
    estimate m^(t) = sv(t) * sigma_h * sqrt(2 ln t), where sv(t) =
    log(t)^alpha/sqrt(D) is the position scale and sigma_h is the score std
    derived from the w_q/w_k column norms. On this problem's data the
    estimate sits within [-55, +43] of the true row max; storing P = exp(s -
    m^) in bf16 (range e^+-88) and accumulating in fp32 PSUM absorbs that
    slack. The shift cancels exactly between numerator and denominator.
  - x is shipped once (chunk-major layout, 8KB contiguous runs per DMA
    descriptor); the position scale is applied on-chip by multiplying the
    q-projection PSUM with a persistent broadcast tile svb = ones^T x sv
    during the PSUM->SBUF copy, so no second pre-scaled copy of x is needed.
  - The attention inner loop keeps the PE dense: the strip normalize chain
    and the c_proj matmuls are deferred closures drained into later strips'
    g-loops (normalize with priority, proj as trickled filler), and the v^T
    computation for batches 2-3 is deferred as filler for the first two
    attention pairs. A dense PE keeps the HAM clock gate at 2.4 GHz.
  - exp is batched two k-tiles at a time over a [128,1024] 2-bank PSUM tile
    to amortize the ~350-cycle ACT instruction overhead.
  - v is computed transposed (stationary w_v, 8 matmuls/chunk) then moved to
    natural [rows, feat] layout with PE transposes.
  - c_proj is emitted output-transposed (stationary w_proj tiles), writing
    outT [C, B*T]; the host transposes after the cross-core sum.
  - softmax denominators are inverted with the fast approximate reciprocal
    (copied to SBUF first: its bitwise seed must see IEEE fp32, not PSUM's
    internal accumulator format).

Layout (per core):
  - xC [chunk, 128, kt, 512] fp16, contraction-major within each chunk.
  - qsT [65, pair, T]: rows 0-63 = q' = sv * (w_q^T x), row 64 = -m^(t).
  - kaT [65, pair, T]: rows 0-63 = k, row 64 = ones. S^T = kaT^T @ qsT gives
    scores with the shift already applied (65-dim augmented contraction).
  - vA [128, pair, kt, 65] bf16: v natural + ones column; PV with moving
    P^T produces y^T plus the softmax denominator in row 64.
"""

import sys

if "/opt/trn_rl_repo" not in sys.path:
    sys.path.insert(0, "/opt/trn_rl_repo")

import math

import numpy as np

# ---------------------------------------------------------------- constants
B, T, C, H, D = 4, 2048, 1024, 16, 64
ALPHA = 2.0
NCORES = 8
HPC = H // NCORES          # heads per core = 2
NP = B * HPC               # (batch, head) pairs per core = 8
BT = B * T                 # 8192 rows
KC = C // 128              # 8 contraction tiles for the qkv projection
CH = 512                   # row chunk / query strip width
NCH = BT // CH             # 16 chunks
QTPB = T // 128            # 16 k-tiles per batch
SPB = T // CH              # 4 query strips per batch
NEG = -1.0e9
DEFER_V_FROM = 8           # chunks >= this defer their v^T to attention filler

_F16 = np.float16


def _build_nc():
    import concourse.mybir as mybir
    from concourse import bacc
    from concourse.masks import make_identity
    from concourse.tile import TileContext

    f16 = mybir.dt.float16
    bf16 = mybir.dt.bfloat16
    f32 = mybir.dt.float32
    EXP = mybir.ActivationFunctionType.Exp

    nc = bacc.Bacc()

    xC = nc.dram_tensor("xC", [NCH, 128, KC, CH], f16, kind="ExternalInput")
    wq = nc.dram_tensor("wq", [C, HPC * D], f16, kind="ExternalInput")
    wk = nc.dram_tensor("wk", [C, HPC * D], f16, kind="ExternalInput")
    wv = nc.dram_tensor("wv", [C, HPC * D], f16, kind="ExternalInput")
    wp = nc.dram_tensor("wp", [HPC * D, C], f16, kind="ExternalInput")
    mneg = nc.dram_tensor("mneg", [NP, T], f16, kind="ExternalInput")
    svr = nc.dram_tensor("svr", [1, T], f16, kind="ExternalInput")
    outT = nc.dram_tensor("outT", [C, BT], f16, kind="ExternalOutput")

    with TileContext(nc) as tc:
        with (
            tc.tile_pool(name="persist", bufs=1) as pp,
            tc.tile_pool(name="xin", bufs=2) as xp,
            tc.tile_pool(name="ptile", bufs=4) as ptp,
            tc.tile_pool(name="small", bufs=2) as sp,
            tc.tile_pool(name="ps2", bufs=2, space="PSUM") as ps2,
            tc.tile_pool(name="ps1", bufs=2, space="PSUM") as ps1,
        ):
            qsT = pp.tile([65, NP, T], f16, tag="qsT")          # q' + (-m^) row
            kaT = pp.tile([65, NP, T], f16, tag="kaT")          # k + ones row
            vA = pp.tile([128, NP, QTPB, 65], bf16, tag="vA")   # v + ones col
            yT = pp.tile([128, BT], f16, tag="yT")              # y^T, 2 heads
            wqs = pp.tile([128, KC, 128], f16, tag="wqs")
            wks = pp.tile([128, KC, 128], f16, tag="wks")
            wvs = pp.tile([128, KC, 128], f16, tag="wvs")
            wps = pp.tile([128, C], f16, tag="wps")
            maskK = pp.tile([128, 128], f32, tag="maskK")       # [k,q]: NEG if q<k
            ones1 = pp.tile([1, 64], bf16, tag="ones1")
            ones1h = pp.tile([1, 64], f16, tag="ones1h")
            svb = pp.tile([64, SPB, CH], f16, tag="svb")        # sv broadcast
            svsb = pp.tile([1, T], f16, tag="svsb")
            identB = pp.tile([128, 128], bf16, tag="identB")

            # ---- init constants
            nc.sync.dma_start(out=wqs, in_=wq[:, :].rearrange("(kt p) n -> p kt n", p=128))
            nc.sync.dma_start(out=wks, in_=wk[:, :].rearrange("(kt p) n -> p kt n", p=128))
            nc.sync.dma_start(out=wvs, in_=wv[:, :].rearrange("(kt p) n -> p kt n", p=128))
            nc.sync.dma_start(out=wps, in_=wp[:, :])
            nc.sync.dma_start(out=svsb, in_=svr[:, :])
            idx = pp.tile([128, 128], mybir.dt.int32, tag="idx")
            nc.gpsimd.iota(idx, pattern=[[1, 128]], base=0, channel_multiplier=-1)
            nc.vector.tensor_scalar(
                out=maskK, in0=idx, scalar1=0, scalar2=float(NEG),
                op0=mybir.AluOpType.is_lt, op1=mybir.AluOpType.mult)
            nc.vector.memset(ones1, 1.0)
            nc.vector.memset(ones1h, 1.0)
            make_identity(nc, identB)
            nc.vector.memset(vA[:, :, :, 64:65], 1.0)
            nc.vector.memset(kaT[64:65, :, :], 1.0)
            nc.sync.dma_start(out=qsT[64:65, :, :], in_=mneg[:, :])
            for s4 in range(SPB):
                svp = ps1.tile([128, CH], f32, tag="po", name="svp")
                nc.tensor.matmul(svp[0:64, :], ones1h,
                                 svsb[0:1, s4 * CH:(s4 + 1) * CH],
                                 start=True, stop=True)
                nc.scalar.copy(svb[:, s4, :], svp[0:64, :])

            # ---- deferred-emission queues
            pending_norm = []
            pending_proj = []

            def drain_pending(k=1):
                while pending_norm:
                    pending_norm.pop(0)()
                if len(pending_proj) > 16:
                    k += 1
                for _ in range(k):
                    if pending_proj:
                        pending_proj.pop(0)()

            # ---- stage A: q/k (+ maybe v^T) projections per 512-row chunk
            def emit_v(n, xt):
                b = n // SPB
                koff = (n % SPB) * (CH // 128)
                psv = ps1.tile([128, CH], f32, tag="po", name="psv")
                for kt in range(KC):
                    nc.tensor.matmul(psv, wvs[:, kt, :], xt[:, kt, :],
                                     start=(kt == 0), stop=(kt == KC - 1))
                vt = sp.tile([128, CH], bf16, tag="vt")
                nc.vector.tensor_copy(vt, psv)
                for j in range(CH // 128):
                    tp = ps1.tile([128, 128], bf16, tag="po", name="tp")
                    nc.tensor.transpose(tp, vt[:, j * 128:(j + 1) * 128], identB)
                    nc.vector.tensor_copy(
                        vA[:, b * HPC:(b + 1) * HPC, koff + j, 0:64],
                        tp[:, :].rearrange("p (h d) -> p h d", h=HPC))

            def emit_chunk(n):
                b, loc = n // SPB, (n % SPB) * CH
                xt = xp.tile([128, KC, CH], f16, tag="xt")
                nc.sync.dma_start(out=xt, in_=xC[n, :, :, :])
                qk = ps2.tile([128, 2 * CH], f32, tag="s")
                for kt in range(KC):
                    nc.tensor.matmul(qk[:, 0:CH], wqs[:, kt, :], xt[:, kt, :],
                                     start=(kt == 0), stop=(kt == KC - 1))
                for kt in range(KC):
                    nc.tensor.matmul(qk[:, CH:2 * CH], wks[:, kt, :], xt[:, kt, :],
                                     start=(kt == 0), stop=(kt == KC - 1))
                for h in range(HPC):
                    pair = b * HPC + h
                    # fused position-scale: q' = q * sv(t)
                    nc.vector.tensor_mul(
                        qsT[0:64, pair, loc:loc + CH],
                        qk[h * 64:(h + 1) * 64, 0:CH], svb[:, n % SPB, :])
                    nc.scalar.copy(
                        kaT[0:64, pair, loc:loc + CH],
                        qk[h * 64:(h + 1) * 64, CH:2 * CH])
                if n < DEFER_V_FROM:
                    emit_v(n, xt)
                else:
                    # defer v^T of later batches into the attention phase as
                    # PE filler for pairs 0-1 (which have no proj ready yet)
                    def unit_dma(n=n):
                        xv = xp.tile([128, KC, CH], f16, tag="xv", name="xv")
                        nc.sync.dma_start(out=xv, in_=xC[n, :, :, :])
                        _defer_x[n] = xv

                    def unit_v(n=n):
                        emit_v(n, _defer_x.pop(n))

                    pending_proj.extend([unit_dma, unit_v])

            _defer_x = {}

            # ---- attention: strip of 512 queries, 2-kt-batched exp pipeline
            def emit_strip(p, qs):
                b, h = p // HPC, p % HPC
                kts = 4 * (qs + 1)
                y_ps = ps1.tile([128, CH], f32, tag="y")
                psSs = {}
                pts = {}

                def offof(kt):
                    return max(0, kt * 128 - qs * CH)

                def emit_S(kt):
                    g, half = kt // 2, kt % 2
                    if half == 0:
                        psSs[g] = ps2.tile([128, 2 * CH], f32, tag="s",
                                           name="psS")
                    psS = psSs[g]
                    off = offof(kt)
                    lo = half * CH + off
                    nc.tensor.matmul(
                        psS[:, lo:(half + 1) * CH],
                        kaT[0:65, p, kt * 128:(kt + 1) * 128],
                        qsT[0:65, p, qs * CH + off:(qs + 1) * CH],
                        start=True, stop=True)
                    if kt >= 4 * qs:
                        nc.vector.tensor_add(
                            psS[:, lo:lo + 128], psS[:, lo:lo + 128], maskK)

                def emit_exp(g):
                    lo = offof(2 * g)
                    pt = ptp.tile([128, 2 * CH], bf16, tag="pt", name="pt")
                    pts[g] = pt
                    nc.scalar.activation(
                        pt[:, lo:2 * CH], psSs[g][:, lo:2 * CH], EXP)

                def emit_PV(kt):
                    g, half = kt // 2, kt % 2
                    off = offof(kt)
                    nc.tensor.matmul(
                        y_ps[0:65, off:CH],
                        vA[:, p, kt, :],
                        pts[g][:, half * CH + off:(half + 1) * CH],
                        start=(kt == 0), stop=(kt == kts - 1))

                gs = kts // 2
                for g in range(gs):
                    emit_S(2 * g)
                    emit_S(2 * g + 1)
                    emit_exp(g)
                    if g > 0:
                        emit_PV(2 * g - 2)
                        emit_PV(2 * g - 1)
                    drain_pending()
                emit_PV(kts - 2)
                emit_PV(kts - 1)

                # normalize: yT[h, rows] = y / denom (denom = row 64)
                state = {}

                def n_rec():
                    # copy denom to SBUF first: the approx reciprocal's
                    # BITWISE_NOT seed must see IEEE fp32 bits, not PSUM's
                    # internal accumulator format
                    dns = sp.tile([1, CH], f32, tag="dns", bufs=3, name="dns")
                    nc.vector.tensor_copy(dns, y_ps[64:65, :])
                    recs = sp.tile([1, CH], f32, tag="recs", bufs=3,
                                   name="recs")
                    nc.vector.reciprocal_approx_fast(out=recs, in_=dns)
                    recb = sp.tile([1, CH], bf16, tag="recb", bufs=3,
                                   name="recb")
                    nc.vector.tensor_copy(recb, recs)
                    state["recb"] = recb

                def n_dbc():
                    dbc = ps1.tile([128, CH], f32, tag="po", name="dbc")
                    nc.tensor.matmul(dbc[0:64, :], ones1, state["recb"],
                                     start=True, stop=True)
                    state["dbc"] = dbc

                def n_mul():
                    dsb = sp.tile([64, CH], bf16, tag="dsb", bufs=3,
                                  name="dsb")
                    nc.vector.tensor_copy(dsb, state["dbc"][0:64, :])
                    nc.vector.tensor_mul(
                        yT[h * 64:(h + 1) * 64,
                           b * T + qs * CH:b * T + (qs + 1) * CH],
                        y_ps[0:64, :], dsb)

                pending_norm.extend([n_rec, n_dbc, n_mul])

            # ---- c_proj, output-transposed: outT[C, rows] per batch,
            # trickled through later strips as PE filler
            def queue_proj(b):
                def unit(nt, rs):
                    def run():
                        po = ps1.tile([128, CH], f32, tag="po", name="po")
                        nc.tensor.matmul(
                            po, wps[:, nt * 128:(nt + 1) * 128],
                            yT[:, b * T + rs * CH:b * T + (rs + 1) * CH],
                            start=True, stop=True)
                        o16 = ptp.tile([128, CH], f16, tag="o16", name="o16")
                        # split the PSUM->SBUF copy across ACT and DVE so the
                        # po slot frees in half the time (the PE's next proj
                        # matmul WAR-waits on this release)
                        nc.scalar.copy(o16[:, 0:CH // 2], po[:, 0:CH // 2])
                        nc.vector.tensor_copy(o16[:, CH // 2:CH],
                                              po[:, CH // 2:CH])
                        nc.sync.dma_start(
                            out=outT[nt * 128:(nt + 1) * 128,
                                     b * T + rs * CH:b * T + (rs + 1) * CH],
                            in_=o16)
                    return run
                for nt in range(C // 128):
                    for rs in range(SPB):
                        pending_proj.append(unit(nt, rs))

            for n in range(NCH):
                emit_chunk(n)
            for p in range(NP):
                for qs in range(SPB):
                    emit_strip(p, qs)
                if p % HPC == HPC - 1:
                    queue_proj(p // HPC)
            drain_pending(len(pending_proj))
    nc.compile()
    return nc


_NC_CACHE = None
TRACE = False           # set by test harness for profiling runs
LAST_RESULT = None      # BassKernelResults of the last run (when TRACE)


def kernel(x, w_attn, w_proj):
    global _NC_CACHE, LAST_RESULT
    from concourse.bass_utils import run_bass_kernel_spmd

    if _NC_CACHE is None:
        _NC_CACHE = _build_nc()
    nc = _NC_CACHE

    x2 = np.asarray(x, dtype=np.float32).reshape(BT, C)
    pos = np.arange(1, T + 1, dtype=np.float64)
    sv = np.log(pos) ** ALPHA / math.sqrt(D)
    # chunk-major x: [chunk, partition, kt, row] with 8KB contiguous runs
    xCm = np.ascontiguousarray(
        x2.reshape(NCH, CH, KC, 128).transpose(0, 3, 2, 1)).astype(_F16)
    wa = np.asarray(w_attn, dtype=np.float32)
    wpj = np.asarray(w_proj, dtype=np.float32)

    # data-independent softmax shift: m^(h, t) = sv(t) * sigma_h * g(t)
    nq = (wa[:, :C] ** 2).sum(axis=0)
    nk = (wa[:, C:2 * C] ** 2).sum(axis=0)
    sigma = np.sqrt((nq * nk).reshape(H, D).sum(axis=1))      # (H,)
    g = np.sqrt(2.0 * np.log(np.clip(pos, 2.0, None)))        # (T,)
    svrow = sv.astype(_F16).reshape(1, T)

    in_maps = []
    for c in range(NCORES):
        h0 = c * HPC
        cols = np.r_[h0 * D:(h0 + HPC) * D]
        mn = np.empty((NP, T), dtype=np.float64)
        for pair in range(NP):
            hg = h0 + pair % HPC
            mn[pair] = -(sv * sigma[hg] * g)
        in_maps.append({
            "xC": xCm,
            "wq": np.ascontiguousarray(wa[:, cols]).astype(_F16),
            "wk": np.ascontiguousarray(wa[:, C + cols]).astype(_F16),
            "wv": np.ascontiguousarray(wa[:, 2 * C + cols]).astype(_F16),
            "wp": np.ascontiguousarray(wpj[cols, :]).astype(_F16),
            "mneg": mn.astype(_F16),
            "svr": svrow,
        })

    res = run_bass_kernel_spmd(
        nc, in_maps, core_ids=list(range(NCORES)), trace=TRACE)
    LAST_RESULT = res
    total = np.zeros((C, BT), dtype=np.float32)
    for r in res.results:
        total += r["outT"].astype(np.float32)
    return np.ascontiguousarray(total.T).reshape(B, T, C)
